# revision 22
# baseline (speedup 1.0000x reference)
"""Bahdanau attention Trainium2 kernel (Bass/Tile), 8-core data-parallel.

Full-input contract: kernel(**inputs) takes the unsharded numpy inputs and
returns (context [128,2048] f32, alpha [128,196] f32) like the reference.

Sharding: batch 128 -> 16 per core; small weight matrices replicated.

Per-core algorithm (B=16, P=196, F=2048, H=1024, A=512), batch pairs:
  X^T tiles via PE transpose-mode (fp32), X loaded once in natural layout;
  enc^T[a, rows] = WeT.T @ X^T accumulated over F (float32r: full rate at
  N=392>=256); tanh on ACT with per-partition bias be[a]+dec[b,a] (bf is
  dropped -- softmax is shift-invariant); scores = WfT_col.T @ tanh (K=A);
  per-batch softmax on partition 0 (fused exp+sum); alpha spread to columns
  with PE transpose; context[1,F] = alphaT.T @ X_natural (N=512, f32r).
"""

import sys
from contextlib import ExitStack

import numpy as np

sys.path.insert(0, "/opt/trn_rl_repo")

import bass_rust
import concourse.bass as bass
import concourse.tile as tile
from concourse import mybir

B_FULL = 128
NPOS = 196
FEAT = 2048
HID = 1024
ATTN = 512
N_CORES = 8

B = B_FULL // N_CORES   # 16
PAIRS = B // 2          # 8
RCOLS = 2 * NPOS        # 392

F32 = mybir.dt.float32
F32R = mybir.dt.float32r
BF16 = mybir.dt.bfloat16

KF = FEAT // 128        # 16
KH = HID // 128         # 8
MA = ATTN // 128        # 4
P0, P1 = 128, NPOS - 128  # 128 + 68

# "f32r": fp32 data, float32r matmuls (full rate, HW numerics ~fp32)
# "f32" : plain fp32 matmuls (4 cyc/row, exact)
# "bf16": enc operands cast to bf16 during transpose copies (full rate)
MM_MODE = "f32r"


def build_kernel(nc: bass.Bass):
    # dtype for tiles feeding the big matmuls: float32r tiles are rounded on
    # write by the producing engine (walrus requires rounded producers).
    if MM_MODE == "bf16":
        mm_dt = BF16
    elif MM_MODE == "f32r":
        mm_dt = F32R
    else:
        mm_dt = F32

    x_dram = nc.dram_tensor("encoder_out", [B, NPOS, FEAT], F32, kind="ExternalInput").ap()
    h_dram = nc.dram_tensor("hidden", [B, HID], F32, kind="ExternalInput").ap()
    we_dram = nc.dram_tensor("We", [ATTN, FEAT], F32, kind="ExternalInput").ap()
    be_dram = nc.dram_tensor("be", [ATTN], F32, kind="ExternalInput").ap()
    wd_dram = nc.dram_tensor("Wd", [ATTN, HID], F32, kind="ExternalInput").ap()
    bd_dram = nc.dram_tensor("bd", [ATTN], F32, kind="ExternalInput").ap()
    wf_dram = nc.dram_tensor("Wf", [1, ATTN], F32, kind="ExternalInput").ap()
    nc.dram_tensor("bf", [1], F32, kind="ExternalInput")  # softmax-invariant

    ctx_dram = nc.dram_tensor("context", [B, FEAT], F32, kind="ExternalOutput").ap()
    alpha_dram = nc.dram_tensor("alpha", [B, NPOS], F32, kind="ExternalOutput").ap()
    alpha_flat = alpha_dram.rearrange("b p -> (b p)")

    with ExitStack() as ctx:
        tc = ctx.enter_context(tile.TileContext(nc))

        const_pool = ctx.enter_context(tc.tile_pool(name="const", bufs=1))
        wtmp_pool = ctx.enter_context(tc.tile_pool(name="wtmp", bufs=1))
        x_pool = ctx.enter_context(tc.tile_pool(name="x", bufs=2))
        xt_pool = ctx.enter_context(tc.tile_pool(name="xt", bufs=1))
        tanh_pool = ctx.enter_context(tc.tile_pool(name="tanh", bufs=2))
        small_pool = ctx.enter_context(tc.tile_pool(name="small", bufs=4))
        out_pool = ctx.enter_context(tc.tile_pool(name="out", bufs=2))
        dram_pool = ctx.enter_context(tc.tile_pool(name="dram", bufs=2, space="DRAM"))

        psum_t = ctx.enter_context(tc.tile_pool(name="psum_t", bufs=2, space="PSUM"))
        psum_enc = ctx.enter_context(tc.tile_pool(name="psum_enc", bufs=3, space="PSUM"))
        psum_sc = ctx.enter_context(tc.tile_pool(name="psum_sc", bufs=1, space="PSUM"))

        # walrus allows at most ONE semaphore wait per Matmult instruction.
        # PE "absorber" NOPs carry any extra cross-engine waits so each
        # matmul/transpose is left with <=1 unobserved dependency.
        last_eng = {"dve": None, "act": None, "pe": None}

        def dve(inst):
            last_eng["dve"] = inst
            return inst

        def act(inst):
            last_eng["act"] = inst
            return inst

        def pe(inst):
            last_eng["pe"] = inst
            return inst

        def absorb(*keys):
            # handled by bacc.compile() (move_matmul_waits_to_ldweights +
            # generate_event_semaphores); kept as a no-op marker.
            return

        ident = const_pool.tile([128, 128], F32)
        nc.gpsimd.memset(ident, 0.0)
        ident_inst = nc.gpsimd.affine_select(
            out=ident, in_=ident, compare_op=mybir.AluOpType.not_equal,
            fill=1.0, base=0, pattern=[[-1, 128]], channel_multiplier=1,
        )

        copy_i = 0

        def cp(out_ap, in_ap):
            nonlocal copy_i
            copy_i += 1
            if copy_i % 2 == 0:
                last_eng["act"] = nc.scalar.activation(
                    out_ap, in_ap, mybir.ActivationFunctionType.Copy
                )
            else:
                last_eng["dve"] = nc.vector.tensor_copy(out_ap, in_ap)

        # ---- weights: WeT_all[p, k, a] = We[a, k*128+p]; same for Wd ----
        weT_all = const_pool.tile([128, KF, ATTN], mm_dt)
        wdT_all = const_pool.tile([128, KH, ATTN], F32)

        for (src, kk, kt_all, width) in (
            (we_dram, KF, weT_all, FEAT),
            (wd_dram, KH, wdT_all, HID),
        ):
            for pa in range(MA):
                w_nat = wtmp_pool.tile([128, FEAT], F32, tag="w_nat")
                nc.sync.dma_start(out=w_nat[:, :width], in_=src[pa * 128:(pa + 1) * 128, :])
                absorb("dve", "act", "pe")
                for k0 in range(0, kk, 8):
                    ptg = psum_t.tile([128, 8, 128], F32, tag="pt")
                    for j in range(8):
                        pe(nc.tensor.transpose(
                            ptg[:, j, :],
                            w_nat[:, (k0 + j) * 128:(k0 + j + 1) * 128],
                            ident,
                        ))
                    cp(kt_all[:, k0:k0 + 8, pa * 128:(pa + 1) * 128], ptg)

        # ---- hidden -> hT[k] [128, B] ------------------------------------
        h_nat = const_pool.tile([B, HID], F32)
        nc.sync.dma_start(out=h_nat, in_=h_dram)
        hT = const_pool.tile([128, KH, B], F32)
        absorb("dve", "act", "pe")
        for k0 in range(0, KH, 8):
            ptg = psum_t.tile([128, 8, B], F32, tag="pt")
            for j in range(8):
                pe(nc.tensor.transpose(
                    ptg[:, j, :],
                    h_nat[:, (k0 + j) * 128:(k0 + j + 1) * 128],
                    ident[:B, :B],
                ))
            cp(hT[:, k0:k0 + 8, :], ptg)

        # ---- bias prep ---------------------------------------------------
        bd_sb = const_pool.tile([128, MA], F32)
        be_sb = const_pool.tile([128, MA], F32)
        nc.sync.dma_start(out=bd_sb, in_=bd_dram.rearrange("(c p) -> p c", p=128))
        nc.sync.dma_start(out=be_sb, in_=be_dram.rearrange("(c p) -> p c", p=128))
        bdbe = const_pool.tile([128, MA], F32)
        dve(nc.vector.tensor_add(bdbe, bd_sb, be_sb))

        wfT_raw = const_pool.tile([128, MA], F32)
        nc.sync.dma_start(out=wfT_raw, in_=wf_dram.rearrange("o (c p) -> p (o c)", p=128))
        wfT = const_pool.tile([128, MA], mm_dt)  # rounded copy for the matmul
        dve(nc.vector.tensor_copy(wfT, wfT_raw))

        # ---- decT[m] + (bd+be) -> biasT [128, MA, B] ---------------------
        absorb("dve", "act", "pe")
        biasT = const_pool.tile([128, MA, B], F32)
        for m in range(MA):
            pd = psum_t.tile([128, B], F32, tag="pt")
            for k in range(KH):
                pe(nc.tensor.matmul(
                    pd,
                    lhsT=wdT_all[:, k, m * 128:(m + 1) * 128],
                    rhs=hT[:, k, :],
                    start=(k == 0),
                    stop=(k == KH - 1),
                ))
            dve(nc.vector.tensor_scalar_add(biasT[:, m, :], pd, bdbe[:, m:m + 1]))

        # ---- main loop ---------------------------------------------------
        x_flat = x_dram.rearrange("b p f -> (b p) f")

        for pair in range(PAIRS):
            b0 = 2 * pair
            x_nat = []
            for bh in range(2):
                xb = x_pool.tile([128, 2, FEAT], F32, tag=f"x_nat{bh}")
                r0 = (b0 + bh) * NPOS
                nc.sync.dma_start(out=xb[:, 0, :], in_=x_flat[r0:r0 + P0, :])
                nc.sync.dma_start(out=xb[:P1, 1, :], in_=x_flat[r0 + P0:r0 + NPOS, :])
                x_nat.append(xb)

            xT = xt_pool.tile([128, KF, RCOLS], mm_dt, tag="xT")
            for bh in range(2):
                for pt_i, rc in ((0, P0), (1, P1)):
                    col = bh * NPOS + pt_i * 128
                    absorb("dve", "act", "pe")
                    for k0 in range(0, KF, 8):
                        ptg = psum_t.tile([128, 8, 128], F32, tag="pt")
                        for j in range(8):
                            pe(nc.tensor.transpose(
                                ptg[:, j, :rc],
                                x_nat[bh][:rc, pt_i, (k0 + j) * 128:(k0 + j + 1) * 128],
                                ident[:rc, :rc],
                            ))
                        cp(xT[:, k0:k0 + 8, col:col + rc], ptg[:, :, :rc])

            # enc^T chunks + tanh
            absorb("dve", "act", "pe")
            tanh_sb = []
            for m in range(MA):
                if m == 3:
                    absorb("act", "pe")
                penc = psum_enc.tile([128, RCOLS], F32, tag="pe")
                for k in range(KF):
                    pe(nc.tensor.matmul(
                        penc,
                        lhsT=weT_all[:, k, m * 128:(m + 1) * 128],
                        rhs=xT[:, k, :],
                        start=(k == 0),
                        stop=(k == KF - 1),
                    ))
                th = tanh_pool.tile([128, RCOLS], mm_dt, tag=f"tanh{m}")
                for bh in range(2):
                    act(nc.scalar.activation(
                        th[:, bh * NPOS:(bh + 1) * NPOS],
                        penc[:, bh * NPOS:(bh + 1) * NPOS],
                        mybir.ActivationFunctionType.Tanh,
                        bias=biasT[:, m, b0 + bh:b0 + bh + 1],
                        scale=1.0,
                    ))
                tanh_sb.append(th)

            # scores [1, RCOLS]
            ps = psum_sc.tile([1, RCOLS], F32, tag="ps")
            for m in range(MA):
                pe(nc.tensor.matmul(
                    ps,
                    lhsT=wfT[:, m:m + 1],
                    rhs=tanh_sb[m],
                    start=(m == 0),
                    stop=(m == MA - 1),
                ))

            # per-batch softmax on partition 0
            alpha_sb = out_pool.tile([1, RCOLS], F32, tag="alpha")
            for bh in range(2):
                seg = slice(bh * NPOS, (bh + 1) * NPOS)
                nmax = small_pool.tile([1, 1], F32, tag="nmax")
                dve(nc.vector.reduce_max(nmax, ps[0:1, seg], axis=mybir.AxisListType.X, negate=True))
                ssum = small_pool.tile([1, 1], F32, tag="ssum")
                act(nc.scalar.activation(
                    alpha_sb[0:1, seg],
                    ps[0:1, seg],
                    mybir.ActivationFunctionType.Exp,
                    bias=nmax[0:1, :],
                    scale=1.0,
                    accum_out=ssum[0:1, :],
                ))
                sinv = small_pool.tile([1, 1], F32, tag="sinv")
                dve(nc.vector.reciprocal(sinv, ssum))
                dve(nc.vector.tensor_scalar_mul(alpha_sb[0:1, seg], alpha_sb[0:1, seg], sinv[0:1, :]))

            nc.sync.dma_start(
                out=alpha_flat[b0 * NPOS:(b0 + 2) * NPOS],
                in_=alpha_sb,
            )

            # context on DVE: broadcast alpha across partitions, then
            # per k-tile fused multiply + free-dim reduce over positions.
            asc = dram_pool.tile([1, RCOLS], F32, tag="asc")
            nc.sync.dma_start(out=asc, in_=alpha_sb)
            ab = out_pool.tile([128, RCOLS], F32, tag="ab")
            nc.gpsimd.dma_start(out=ab, in_=asc.to_broadcast([128, RCOLS]))
            for bh in range(2):
                seg = slice(bh * NPOS, (bh + 1) * NPOS)
                ctxT = out_pool.tile([128, KF], F32, tag="ctxT")
                for k in range(KF):
                    # tensor_tensor_reduce hangs on HW; split mult (DVE) +
                    # copy-accumulate (ACT) instead.
                    scr = out_pool.tile([128, NPOS], F32, tag="ttr_scr")
                    dve(nc.vector.tensor_tensor(
                        out=scr,
                        in0=xT[:, k, seg].bitcast(F32) if mm_dt == F32R else xT[:, k, seg],
                        in1=ab[:, seg],
                        op=mybir.AluOpType.mult,
                    ))
                    scr2 = out_pool.tile([128, NPOS], F32, tag="ttr_scr2")
                    act(nc.scalar.activation(
                        scr2, scr, mybir.ActivationFunctionType.Copy,
                        accum_out=ctxT[:, k:k + 1],
                    ))
                nc.sync.dma_start(
                    out=ctx_dram[b0 + bh:b0 + bh + 1, :].rearrange("o (k p) -> p (o k)", p=128),
                    in_=ctxT,
                )


_CACHE = {}


def _get_nc():
    if "nc" not in _CACHE:
        from concourse import bacc

        nc = bacc.Bacc("TRN2", target_bir_lowering=False, debug=False)
        build_kernel(nc)
        nc.compile()
        _CACHE["nc"] = nc
    return _CACHE["nc"]


def shard_inputs(inputs):
    bs = B_FULL // N_CORES
    in_maps = []
    for c in range(N_CORES):
        in_maps.append({
            "encoder_out": np.ascontiguousarray(inputs["encoder_out"][c * bs:(c + 1) * bs]),
            "hidden": np.ascontiguousarray(inputs["hidden"][c * bs:(c + 1) * bs]),
            "We": np.asarray(inputs["We"]), "be": np.asarray(inputs["be"]),
            "Wd": np.asarray(inputs["Wd"]), "bd": np.asarray(inputs["bd"]),
            "Wf": np.asarray(inputs["Wf"]), "bf": np.asarray(inputs["bf"]),
        })
    return in_maps


def kernel(**inputs):
    from concourse import bass_utils

    nc = _get_nc()
    in_maps = shard_inputs(inputs)
    res = bass_utils.run_bass_kernel_spmd(nc, in_maps, core_ids=list(range(N_CORES)))
    context = np.concatenate([res.results[c]["context"] for c in range(N_CORES)], axis=0)
    alpha = np.concatenate([res.results[c]["alpha"] for c in range(N_CORES)], axis=0)
    return context, alpha
